# revision 1
# baseline (speedup 1.0000x reference)
"""GCGRU (graph-conv GRU encoder/decoder) on 8 Trainium2 NeuronCores.

Sharding: data-parallel over batch B=64 -> 8 per core (per the hint);
G [3,1024,1024] and all weights replicated on every core. The T=12
encoder + 12-step decoder time loop runs sequentially on-device; the
only host traffic is the initial shard scatter and final gather.
"""
import numpy as np
import jax
import jax.numpy as jnp
from functools import partial

N = 1024   # nodes
K = 3      # cheb supports
H = 64     # hidden
C = 1      # in/out dim
T = 12     # encoder steps
HOR = 12   # decoder horizon
B = 64     # batch
M = 8      # cores

_DIN = C + H


def _gcn(G, x, W, b):
    bb, nn, cc = x.shape
    sup = jnp.einsum('kij,bjc->bikc', G, x)
    return sup.reshape(bb, nn, -1) @ W + b


def _cell(G, x_t, h, Wg, bg, Wu, bu):
    comb = jnp.concatenate([x_t, h], axis=-1)
    z, r = jnp.split(jax.nn.sigmoid(_gcn(G, comb, Wg, bg)), 2, axis=-1)
    n = jnp.tanh(_gcn(G, jnp.concatenate([x_t, r * h], axis=-1), Wu, bu))
    return z * n + (1.0 - z) * h


@partial(jax.pmap, axis_name='i',
         in_axes=(0, None, None, None, None, None, None, None, None, None, None, None))
def _run(x, G, enc_Wg, enc_bg, enc_Wu, enc_bu,
         dec_Wg, dec_bg, dec_Wu, dec_bu, proj_W, proj_b):
    bb = x.shape[0]
    h0 = jnp.zeros((bb, N, H), x.dtype)

    def enc_step(h, x_t):
        return _cell(G, x_t, h, enc_Wg, enc_bg, enc_Wu, enc_bu), None

    h, _ = jax.lax.scan(enc_step, h0, x.transpose(1, 0, 2, 3))

    y0 = jnp.zeros((bb, N, C), x.dtype)

    def dec_step(carry, _):
        h, y = carry
        h = _cell(G, y, h, dec_Wg, dec_bg, dec_Wu, dec_bu)
        out = h @ proj_W + proj_b
        return (h, out), out

    _, outs = jax.lax.scan(dec_step, (h, y0), None, length=HOR)
    return outs.transpose(1, 0, 2, 3)


def kernel(**inputs):
    x = np.asarray(inputs['x'], dtype=np.float32)
    xs = jnp.asarray(x.reshape(M, B // M, T, N, C))
    args = tuple(jnp.asarray(np.asarray(inputs[k], dtype=np.float32)) for k in
                 ('G', 'enc_Wg', 'enc_bg', 'enc_Wu', 'enc_bu',
                  'dec_Wg', 'dec_bg', 'dec_Wu', 'dec_bu', 'proj_W', 'proj_b'))
    out = _run(xs, *args)
    return np.asarray(out).reshape(B, HOR, N, C).astype(np.float32)



# revision 29
# speedup vs baseline: 1.3002x; 1.3002x over previous
"""GCGRU (graph-conv GRU encoder/decoder) on 8 Trainium2 NeuronCores.

Fused Bass/Tile kernel, data-parallel over batch B=64 -> 8 per core.
All state (h, graph supports, weights) stays resident in SBUF across the
whole 24-step recurrence; per call the only device traffic is the x-derived
rows in and y out.

Formulation notes (exploits G[0] == I, verified at runtime):
  GCN(v) = sum_k (G_k @ v) @ W_k. The big matmuls G_k @ h are run on the PE
  in "transposed output" orientation (stationary = h tile, moving = G_k^T
  rows) so the result lands directly as S_k^T rows, which is the layout the
  second-stage (feature-contraction) matmuls need -- no explicit transposes
  of S_k. The decoder's y feedback (y = h @ Wp + bp) is folded into the
  second-stage weight tables on the host, so y never exists on device except
  as an extra output column. Encoder x contributions (G_k @ x_t) are
  precomputed on the host and streamed in as 3 rows per step.
"""
import numpy as np

N = 1024   # nodes
K = 3      # cheb supports
H = 64     # hidden
C = 1      # in/out dim
T = 12     # encoder steps
HOR = 12   # decoder horizon
B = 64     # batch
M = 8      # cores
BB = B // M  # batch per core (8)
IC = N // 128  # node chunks (8)

_BUILT = None  # cached (consts_key, nc) program


# ---------------------------------------------------------------- host prep

def _split_gcn_w(W, dout):
    """W: [K*(C+H), dout] in k-major order -> per-k (wx [dout], Wh [H, dout])."""
    din = C + H
    wx, Wh = [], []
    for k in range(K):
        blk = W[k * din:(k + 1) * din]
        wx.append(blk[0].astype(np.float32))
        Wh.append(blk[1:].astype(np.float32))
    return wx, Wh


def _prep_consts(inputs):
    import ml_dtypes
    bf = ml_dtypes.bfloat16
    G = np.asarray(inputs['G'], np.float32)
    eg_wx, eg_Wh = _split_gcn_w(np.asarray(inputs['enc_Wg'], np.float32), 2 * H)
    eu_wx, eu_Wh = _split_gcn_w(np.asarray(inputs['enc_Wu'], np.float32), H)
    dg_wx, dg_Wh = _split_gcn_w(np.asarray(inputs['dec_Wg'], np.float32), 2 * H)
    du_wx, du_Wh = _split_gcn_w(np.asarray(inputs['dec_Wu'], np.float32), H)
    bg_e = np.asarray(inputs['enc_bg'], np.float32)
    bu_e = np.asarray(inputs['enc_bu'], np.float32)
    bg_d = np.asarray(inputs['dec_bg'], np.float32)
    bu_d = np.asarray(inputs['dec_bu'], np.float32)
    wp = np.asarray(inputs['proj_W'], np.float32)[:, 0]  # [H]
    bp = float(np.asarray(inputs['proj_b'], np.float32)[0])

    c = {}
    # G_k^T tiles for the big matmuls: gt[p, k, jc, i] = G_{k+1}[i, jc*128+p]
    gt = np.empty((128, 2, IC, N), np.float32)
    for k in range(2):
        Gk = G[k + 1]                      # [i, j]
        # gt[p, k, jc, i] = Gk[i, jc*128+p]
        gt[:, k] = Gk.T.reshape(IC, 128, N).transpose(1, 0, 2)
    c['gt'] = gt.astype(bf)

    # g1 rows (G_k @ ones), broadcast over batch: [2, IC*BB*128]
    g1 = G[1:].sum(axis=2)                 # [2, N]
    g1st = np.broadcast_to(
        g1.reshape(2, IC, 1, 128), (2, IC, BB, 128)).reshape(2, IC * BB * 128)
    c['g1st'] = np.ascontiguousarray(g1st).astype(bf)

    def vs(*rows):
        return np.ascontiguousarray(np.vstack(rows)).astype(bf)

    z64 = np.zeros((64, 2 * H), np.float32)
    # ---- encoder gates: lhsT rows [hT(64); S1hT(64)] and [S2hT(64); ones;
    #      xT; Xs1T; Xs2T]
    c['enc_g_a'] = vs(eg_Wh[0], eg_Wh[1])                      # [128, 128]
    c['enc_g_b'] = vs(eg_Wh[2], bg_e[None],
                      eg_wx[0][None], eg_wx[1][None], eg_wx[2][None])  # [68,128]
    # ---- encoder update: S2a rows [rhT; U1hT], S2b rows [U2hT; ones],
    #      STb rows [S2hT(zero-weight); ones(zero); xT; Xs1T; Xs2T]
    c['enc_u_a'] = vs(eu_Wh[0], eu_Wh[1])                      # [128, 64]
    c['enc_u_b'] = vs(eu_Wh[2], bu_e[None])                    # [65, 64]
    c['enc_u_x'] = vs(np.zeros((65, H), np.float32),
                      eu_wx[0][None], eu_wx[1][None], eu_wx[2][None])  # [68, 64]
    # ---- decoder step 0 (y = 0)
    c['dec0_g_a'] = vs(dg_Wh[0], dg_Wh[1])
    c['dec0_g_b'] = vs(dg_Wh[2], bg_d[None], np.zeros((2, 2 * H), np.float32))
    c['dec0_u_a'] = vs(du_Wh[0], du_Wh[1])
    c['dec0_u_b'] = vs(du_Wh[2], bu_d[None])
    # ---- decoder steps >= 1: y = h @ Wp + bp folded in; extra y output col
    c['dec_g_a'] = vs(dg_Wh[0] + np.outer(wp, dg_wx[0]),
                      dg_Wh[1] + np.outer(wp, dg_wx[1]))
    c['dec_g_b'] = vs(dg_Wh[2] + np.outer(wp, dg_wx[2]),
                      (bg_d + bp * dg_wx[0])[None],
                      (bp * dg_wx[1])[None], (bp * dg_wx[2])[None])  # [67,128]

    def ycol(mat, col):
        return np.concatenate([mat, col[:, None]], axis=1)

    c['dec_u_a'] = vs(ycol(np.vstack([du_Wh[0], du_Wh[1]]),
                           np.zeros(128, np.float32)))          # [128, 65]
    ub = np.vstack([du_Wh[2], (bu_d + bp * du_wx[0])[None]])    # [65, 64]
    ubc = np.zeros(65, np.float32); ubc[64] = bp
    c['dec_u_b'] = vs(ycol(ub, ubc))                            # [65, 65]
    uga = np.vstack([np.outer(wp, du_wx[0]), np.outer(wp, du_wx[1])])
    ugac = np.concatenate([wp, np.zeros(64, np.float32)])
    c['dec_u_ga'] = vs(ycol(uga, ugac))                         # [128, 65]
    ugb = np.vstack([np.outer(wp, du_wx[2]), np.zeros((1, H), np.float32),
                     (bp * du_wx[1])[None], (bp * du_wx[2])[None]])
    c['dec_u_gb'] = vs(ycol(ugb, np.zeros(67, np.float32)))     # [67, 65]
    # phantom y emission
    c['wp_col'] = np.ascontiguousarray(wp[:, None]).astype(bf)  # [64, 1]
    c['bp'] = bp
    return c


def _prep_xst(inputs):
    """Per-core runtime input: [T, 3, IC*BB*128] bf16 rows
    (xT, (G1 x)^T, (G2 x)^T)."""
    import ml_dtypes
    bf = ml_dtypes.bfloat16
    G = np.asarray(inputs['G'], np.float32)
    x = np.asarray(inputs['x'], np.float32)[..., 0]   # [B, T, N]
    xf = np.ascontiguousarray(x.reshape(B * T, N))
    xs = np.stack([xf @ G[1].T, xf @ G[2].T]).reshape(2, B, T, N)
    rows = np.concatenate([x[None], xs], axis=0)      # [3, B, T, N]
    out = []
    for cix in range(M):
        r = rows[:, cix * BB:(cix + 1) * BB]          # [3, BB, T, N]
        # -> [T, 3, IC, BB, 128]
        r = r.reshape(3, BB, T, IC, 128).transpose(2, 0, 3, 1, 4)
        out.append(np.ascontiguousarray(r.reshape(T, 3, IC * BB * 128)).astype(bf))
    return out


# ------------------------------------------------------------ program build

def _build_program(consts):
    import concourse.bass as bass
    import concourse.tile as tile
    from concourse import bacc, mybir
    from concourse.masks import make_identity
    from contextlib import ExitStack

    F32 = mybir.dt.float32
    BF16 = mybir.dt.bfloat16
    AF = mybir.ActivationFunctionType

    nc = bacc.Bacc("TRN2", target_bir_lowering=False, debug=False)
    xst_d = nc.dram_tensor("xst", [T, 3, IC * BB * 128], BF16,
                           kind="ExternalInput")
    y_d = nc.dram_tensor("y", [HOR * BB, N], F32, kind="ExternalOutput")
    cd = {k: nc.inline_tensor(v, f"c_{k}") for k, v in consts.items()
          if isinstance(v, np.ndarray)}
    bp = consts['bp']

    with tile.TileContext(nc) as tc, ExitStack() as ctx:
        cpool = ctx.enter_context(tc.tile_pool(name="const", bufs=1))
        spool = ctx.enter_context(tc.tile_pool(name="state", bufs=1))
        wpool = ctx.enter_context(tc.tile_pool(name="work", bufs=3))
        psA = ctx.enter_context(tc.tile_pool(name="psA", bufs=4, space="PSUM"))
        psB = ctx.enter_context(tc.tile_pool(name="psB", bufs=2, space="PSUM"))
        psT = ctx.enter_context(tc.tile_pool(name="psT", bufs=2, space="PSUM"))

        # ---- persistent tiles
        gt = cpool.tile([128, 2, IC, N], BF16)
        g1s = cpool.tile([2, IC * BB * 128], BF16)
        ident = cpool.tile([128, 128], BF16)
        identf = cpool.tile([128, 128], F32)
        W = {}
        for k, v in consts.items():
            if isinstance(v, np.ndarray) and k not in ('gt', 'g1st'):
                W[k] = cpool.tile(list(v.shape), BF16, name=f"w_{k}")
                nc.sync.dma_start(W[k][:], cd[k].ap())
        nc.sync.dma_start(gt[:], cd['gt'].ap())
        nc.sync.dma_start(g1s[:], cd['g1st'].ap())
        make_identity(nc, ident[:])
        make_identity(nc, identf[:])

        ha = spool.tile([128, IC, BB, H], F32)     # h master
        hb = spool.tile([128, IC, BB, H], BF16)    # h bf16 (matmul stationary)
        rhb = spool.tile([128, IC, BB, H], BF16)   # r*h bf16
        STa = spool.tile([128, IC, BB, 128], BF16)  # rows 0:64 hT, 64:128 S1hT
        STb = spool.tile([68, IC, BB, 128], BF16)   # 0:64 S2hT, 64 ones, 65:68 x
        S2a = spool.tile([128, IC, BB, 128], BF16)  # 0:64 rhT, 64:128 U1hT
        S2b = spool.tile([65, IC, BB, 128], BF16)   # 0:64 U2hT, 64 ones
        zr = spool.tile([128, IC, BB, 2 * H], F32)
        yall = spool.tile([128, IC, HOR, BB], F32)
        yT = spool.tile([HOR * BB, N], F32)

        for t_ in (ha, hb, STa, S2a):
            nc.gpsimd.memset(t_[:], 0.0)
        nc.gpsimd.memset(STb[0:64, :, :, :], 0.0)
        nc.gpsimd.memset(S2b[0:64, :, :, :], 0.0)
        nc.gpsimd.memset(STb[64:68, :, :, :], 0.0)
        nc.gpsimd.memset(STb[64:65, :, :, :], 1.0)
        nc.gpsimd.memset(S2b[64:65, :, :, :], 1.0)

        def bigmm(stat, dst_hi, dst_lo):
            """S_k^T = (G_k @ v)^T for k=1,2 into dst rows (bf16)."""
            for ih in range(2):
                for bp_ in range(4):
                    for k in range(2):
                        ps = psA.tile([128, 512], F32, tag="big")
                        for jc in range(IC):
                            nc.tensor.matmul(
                                ps[:],
                                stat[:, jc, 2 * bp_:2 * bp_ + 2, :],
                                gt[:, k, jc, ih * 512:(ih + 1) * 512],
                                start=(jc == 0), stop=(jc == IC - 1))
                        dst = dst_hi if k == 0 else dst_lo
                        rb = 64 if k == 0 else 0
                        for par in range(2):
                            b = 2 * bp_ + par
                            src = ps[par * 64:(par + 1) * 64, :].rearrange(
                                "p (a n) -> p a n", n=128)
                            # nc.any: let the scheduler route each drain to
                            # whichever of ACT/DVE is idle (measured better
                            # than any static split)
                            nc.any.tensor_copy(
                                dst[rb:rb + 64, ih * 4:(ih + 1) * 4, b, :], src)

        def stage2(mms, cols, out_cb):
            """mms: list of (lhsT_fn(ic,b) -> AP, W tile). out [128, 4*cols]
            per (ic, bgroup); out_cb(ic, bg, ps)."""
            for bg in range(2):
                for ic in range(IC):
                    ps = psB.tile([128, 4 * cols], F32, tag="s2")
                    for bi in range(4):
                        b = bg * 4 + bi
                        o = ps[:, bi * cols:(bi + 1) * cols]
                        for i, (lf, Wt) in enumerate(mms):
                            nc.tensor.matmul(o, lf(ic, b), Wt[:],
                                             start=(i == 0),
                                             stop=(i == len(mms) - 1))
                    out_cb(ic, bg, ps)

        def transpose_to(src, dst, dst_rb):
            """src [128, IC, BB, 64] bf16 -> dst[dst_rb:dst_rb+64] rows (f on
            partitions), per (ic, bgroup)."""
            for bg in range(2):
                for ic in range(IC):
                    pst = psT.tile([128, 256], BF16, tag="s2")
                    for bl in range(2):
                        b0 = bg * 4 + 2 * bl
                        nc.tensor.transpose(
                            pst[:, bl * 128:(bl + 1) * 128],
                            src[:, ic, b0:b0 + 2, :], ident[:])
                    for par in range(2):
                        b0 = bg * 4 + par
                        s = pst[par * 64:(par + 1) * 64, :].rearrange(
                            "p (a n) -> p a n", n=128)
                        nc.any.tensor_copy(
                            dst[dst_rb:dst_rb + 64, ic, b0:b0 + 3:2, :], s)

        def gates_cb(ic, bg, ps):
            nc.scalar.activation(zr[:, ic, bg * 4:bg * 4 + 4, :], ps[:],
                                 AF.Sigmoid)
            nc.vector.tensor_tensor(
                rhb[:, ic, bg * 4:bg * 4 + 4, :],
                zr[:, ic, bg * 4:bg * 4 + 4, H:2 * H],
                ha[:, ic, bg * 4:bg * 4 + 4, :],
                mybir.AluOpType.mult)

        def update_cb_factory(cols, d):
            def update_cb(ic, bg, ps):
                bs = slice(bg * 4, bg * 4 + 4)
                pv = ps.rearrange("p (a c) -> p a c", c=cols)
                nt = wpool.tile([128, 4, H], F32, tag="nt")
                nc.scalar.activation(nt[:], pv[:, :, 0:H], AF.Tanh)
                if cols == 65:
                    nc.any.tensor_copy(yall[:, ic, d - 1, bs], pv[:, :, 64])
                dt_ = wpool.tile([128, 4, H], F32, tag="dt")
                nc.vector.tensor_tensor(dt_[:], nt[:], ha[:, ic, bs, :],
                                        mybir.AluOpType.subtract)
                mt = wpool.tile([128, 4, H], F32, tag="mt")
                nc.vector.tensor_tensor(mt[:], zr[:, ic, bs, 0:H], dt_[:],
                                        mybir.AluOpType.mult)
                nc.vector.tensor_tensor(ha[:, ic, bs, :], ha[:, ic, bs, :],
                                        mt[:], mybir.AluOpType.add)
                nc.any.tensor_copy(hb[:, ic, bs, :], ha[:, ic, bs, :])
            return update_cb

        def step(g_mms, u_mms, ucols, d):
            # h' of the previous step is transposed AFTER this step's first
            # big-matmul phase is emitted: the PE chews on bigmm (27us) while
            # ACT/DVE finish the previous step's h' elementwise chain, instead
            # of stalling on it at the step boundary.
            bigmm(hb, STa, STb)
            transpose_to(hb, STa, 0)
            stage2(g_mms, 2 * H, gates_cb)
            transpose_to(rhb, S2a, 0)
            bigmm(rhb, S2a, S2b)
            stage2(u_mms, ucols, update_cb_factory(ucols, d))

        # ---------------- encoder
        for t in range(T):
            nc.sync.dma_start(STb[65:68, :, :, :].rearrange(
                "p a b n -> p (a b n)"), xst_d.ap()[t])
            g_mms = [(lambda ic, b: STa[:, ic, b, :], W['enc_g_a']),
                     (lambda ic, b: STb[0:68, ic, b, :], W['enc_g_b'])]
            u_mms = [(lambda ic, b: S2a[:, ic, b, :], W['enc_u_a']),
                     (lambda ic, b: S2b[0:65, ic, b, :], W['enc_u_b']),
                     (lambda ic, b: STb[0:68, ic, b, :], W['enc_u_x'])]
            step(g_mms, u_mms, H, 0)

        # ---------------- decoder
        nc.sync.dma_start(STb[65:67, :, :, :].rearrange(
            "p a b n -> p (a b n)"), g1s[:])
        for d in range(HOR):
            if d == 0:
                g_mms = [(lambda ic, b: STa[:, ic, b, :], W['dec0_g_a']),
                         (lambda ic, b: STb[0:67, ic, b, :], W['dec0_g_b'])]
                u_mms = [(lambda ic, b: S2a[:, ic, b, :], W['dec0_u_a']),
                         (lambda ic, b: S2b[0:65, ic, b, :], W['dec0_u_b'])]
                step(g_mms, u_mms, H, d)
            else:
                g_mms = [(lambda ic, b: STa[:, ic, b, :], W['dec_g_a']),
                         (lambda ic, b: STb[0:67, ic, b, :], W['dec_g_b'])]
                u_mms = [(lambda ic, b: S2a[:, ic, b, :], W['dec_u_a']),
                         (lambda ic, b: S2b[0:65, ic, b, :], W['dec_u_b']),
                         (lambda ic, b: STa[:, ic, b, :], W['dec_u_ga']),
                         (lambda ic, b: STb[0:67, ic, b, :], W['dec_u_gb'])]
                step(g_mms, u_mms, H + 1, d)

        # phantom step: emit y_12 = h_12 @ Wp + bp
        transpose_to(hb, STa, 0)
        for ic in range(IC):
            ps = psB.tile([128, BB], F32, tag="s2")
            for b in range(BB):
                nc.tensor.matmul(ps[:, b:b + 1], STa[0:64, ic, b, :],
                                 W['wp_col'][:], start=True, stop=True)
            nc.scalar.activation(yall[:, ic, HOR - 1, :], ps[:],
                                 AF.Identity, bias=bp)

        # transpose y to [(t,b), n] and store
        for ic in range(IC):
            pst = psB.tile([HOR * BB, 128], F32, tag="s2")
            nc.tensor.transpose(pst[:], yall[:, ic, :, :].rearrange(
                "p a b -> p (a b)"), identf[:])
            nc.any.tensor_copy(yT[:, ic * 128:(ic + 1) * 128], pst[:])
        nc.sync.dma_start(y_d.ap(), yT[:])

    nc.compile()
    return nc


def _make_runner(nc):
    """Build a persistently-jitted executor for the program (the stock
    run_bass_kernel_spmd path re-jits a fresh closure every call)."""
    import jax
    import jax.numpy as jnp
    from jax.experimental.shard_map import shard_map
    from jax.sharding import Mesh, PartitionSpec, NamedSharding
    from concourse import bass2jax, mybir

    bass2jax.install_neuronx_cc_hook()
    assert nc.dbg_addr is None
    pid_name = (nc.partition_id_tensor.name
                if nc.partition_id_tensor is not None else None)

    in_names, out_names, out_avals, zero_shapes = [], [], [], []
    for alloc in nc.m.functions[0].allocations:
        if not isinstance(alloc, mybir.MemoryLocationSet):
            continue
        name = alloc.memorylocations[0].name
        if alloc.kind == "ExternalInput":
            if name != pid_name:
                in_names.append(name)
        elif alloc.kind == "ExternalOutput":
            out_names.append(name)
            shape = tuple(alloc.tensor_shape)
            dtype = mybir.dt.np(alloc.dtype)
            out_avals.append(jax.core.ShapedArray(shape, dtype))
            zero_shapes.append((shape, dtype))
    n_params = len(in_names)
    all_names = list(in_names + out_names)
    if pid_name is not None:
        all_names.append(pid_name)
    all_names = tuple(all_names)

    def _body(*args):
        operands = list(args)
        if pid_name is not None:
            operands.append(bass2jax.partition_id_tensor())
        outs = bass2jax._bass_exec_p.bind(
            *operands,
            out_avals=tuple(out_avals),
            in_names=all_names,
            out_names=tuple(out_names),
            lowering_input_output_aliases=(),
            sim_require_finite=True,
            sim_require_nnan=True,
            nc=nc,
        )
        return tuple(outs)

    devices = jax.devices()[:M]
    mesh = Mesh(np.asarray(devices), ("core",))
    spec = PartitionSpec("core")
    n_outs = len(out_names)
    fn = jax.jit(
        shard_map(_body, mesh=mesh,
                  in_specs=(spec,) * (n_params + n_outs),
                  out_specs=(spec,) * n_outs, check_rep=False),
        donate_argnums=tuple(range(n_params, n_params + n_outs)),
        keep_unused=True)
    sh = NamedSharding(mesh, spec)
    zfns = [jax.jit(lambda s=s, d=d: jnp.zeros((M * s[0],) + s[1:], d),
                    out_shardings=sh) for (s, d) in zero_shapes]

    def run(per_core_inputs):
        """per_core_inputs: list (per core) of dict name->np.ndarray.
        Returns list of dict name->np.ndarray."""
        cat = [np.concatenate([np.asarray(per_core_inputs[c][nm])
                               for c in range(M)], axis=0)
               for nm in in_names]
        outs = fn(*cat, *[z() for z in zfns])
        res = []
        for c in range(M):
            d = {}
            for i, nm in enumerate(out_names):
                a = np.asarray(outs[i])
                d[nm] = a.reshape(M, a.shape[0] // M, *a.shape[1:])[c]
            res.append(d)
        return res

    return run


# ------------------------------------------------------------------ fallback

def _kernel_jax(inputs):
    import jax
    import jax.numpy as jnp
    from functools import partial

    def _gcn(G, x, Wt, b):
        bb, nn, cc = x.shape
        sup = jnp.einsum('kij,bjc->bikc', G, x)
        return sup.reshape(bb, nn, -1) @ Wt + b

    def _cell(G, x_t, h, Wg, bg, Wu, bu):
        comb = jnp.concatenate([x_t, h], axis=-1)
        z, r = jnp.split(jax.nn.sigmoid(_gcn(G, comb, Wg, bg)), 2, axis=-1)
        n = jnp.tanh(_gcn(G, jnp.concatenate([x_t, r * h], axis=-1), Wu, bu))
        return z * n + (1.0 - z) * h

    @partial(jax.pmap, axis_name='i',
             in_axes=(0,) + (None,) * 11)
    def _run(x, G, enc_Wg, enc_bg, enc_Wu, enc_bu,
             dec_Wg, dec_bg, dec_Wu, dec_bu, proj_W, proj_b):
        bb = x.shape[0]
        h0 = jnp.zeros((bb, N, H), x.dtype)

        def enc_step(h, x_t):
            return _cell(G, x_t, h, enc_Wg, enc_bg, enc_Wu, enc_bu), None

        h, _ = jax.lax.scan(enc_step, h0, x.transpose(1, 0, 2, 3))
        y0 = jnp.zeros((bb, N, C), x.dtype)

        def dec_step(carry, _):
            h, y = carry
            h = _cell(G, y, h, dec_Wg, dec_bg, dec_Wu, dec_bu)
            out = h @ proj_W + proj_b
            return (h, out), out

        _, outs = jax.lax.scan(dec_step, (h, y0), None, length=HOR)
        return outs.transpose(1, 0, 2, 3)

    import jax.numpy as jnp
    x = np.asarray(inputs['x'], dtype=np.float32)
    xs = jnp.asarray(x.reshape(M, B // M, T, N, C))
    args = tuple(jnp.asarray(np.asarray(inputs[k], dtype=np.float32)) for k in
                 ('G', 'enc_Wg', 'enc_bg', 'enc_Wu', 'enc_bu',
                  'dec_Wg', 'dec_bg', 'dec_Wu', 'dec_bu', 'proj_W', 'proj_b'))
    out = _run(xs, *args)
    return np.asarray(out).reshape(B, HOR, N, C).astype(np.float32)


# ------------------------------------------------------------------- kernel

def _structure_ok(inputs):
    try:
        x = np.asarray(inputs['x'])
        G = np.asarray(inputs['G'])
        if x.shape != (B, T, N, C) or G.shape != (K, N, N):
            return False
        return bool(np.abs(G[0] - np.eye(N, dtype=np.float32)).max() < 1e-6)
    except Exception:
        return False


def kernel(**inputs):
    global _BUILT
    if not _structure_ok(inputs):
        return _kernel_jax(inputs)
    try:
        import hashlib
        hsh = hashlib.sha256()
        for k in ('G', 'enc_Wg', 'enc_bg', 'enc_Wu', 'enc_bu', 'dec_Wg',
                  'dec_bg', 'dec_Wu', 'dec_bu', 'proj_W', 'proj_b'):
            hsh.update(np.ascontiguousarray(
                np.asarray(inputs[k], np.float32)).tobytes())
        key = hsh.hexdigest()
        if _BUILT is None or _BUILT[0] != key:
            consts = _prep_consts(inputs)
            nc = _build_program(consts)
            _BUILT = (key, _make_runner(nc))
        run = _BUILT[1]
        xsts = _prep_xst(inputs)
        res = run([{"xst": xsts[cix]} for cix in range(M)])
        out = np.empty((B, HOR, N, C), np.float32)
        for cix in range(M):
            yc = res[cix]["y"].reshape(HOR, BB, N)
            out[cix * BB:(cix + 1) * BB] = \
                yc.transpose(1, 0, 2).reshape(BB, HOR, N, C)
        return out
    except Exception:
        _BUILT = None
        return _kernel_jax(inputs)


# revision 37
# speedup vs baseline: 2.4531x; 1.8866x over previous
"""GCGRU (graph-conv GRU encoder/decoder) on 8 Trainium2 NeuronCores.

Fused Bass/Tile kernel, data-parallel over batch B=64 -> 8 per core.
All state (h, graph supports, weights) stays resident in SBUF across the
whole 24-step recurrence; per call the only device traffic is the x-derived
rows in and y out.

Formulation notes (exploits G[0] == I, verified at runtime):
  GCN(v) = sum_k (G_k @ v) @ W_k. The big matmuls G_k @ h are run on the PE
  in "transposed output" orientation (stationary = h tile, moving = G_k^T
  rows) so the result lands directly as S_k^T rows, which is the layout the
  second-stage (feature-contraction) matmuls need -- no explicit transposes
  of S_k. The decoder's y feedback (y = h @ Wp + bp) is folded into the
  second-stage weight tables on the host, so y never exists on device except
  as an extra output column. Encoder x contributions (G_k @ x_t) are
  precomputed on the host and streamed in as 3 rows per step.
"""
import numpy as np

N = 1024   # nodes
K = 3      # cheb supports
H = 64     # hidden
C = 1      # in/out dim
T = 12     # encoder steps
HOR = 12   # decoder horizon
B = 64     # batch
M = 8      # cores
BB = B // M  # batch per core (8)
IC = N // 128  # node chunks (8)

_BUILT = None  # cached (consts_key, nc) program


# ---------------------------------------------------------------- host prep

def _split_gcn_w(W, dout):
    """W: [K*(C+H), dout] in k-major order -> per-k (wx [dout], Wh [H, dout])."""
    din = C + H
    wx, Wh = [], []
    for k in range(K):
        blk = W[k * din:(k + 1) * din]
        wx.append(blk[0].astype(np.float32))
        Wh.append(blk[1:].astype(np.float32))
    return wx, Wh


def _prep_consts(inputs):
    import ml_dtypes
    bf = ml_dtypes.bfloat16
    G = np.asarray(inputs['G'], np.float32)
    eg_wx, eg_Wh = _split_gcn_w(np.asarray(inputs['enc_Wg'], np.float32), 2 * H)
    eu_wx, eu_Wh = _split_gcn_w(np.asarray(inputs['enc_Wu'], np.float32), H)
    dg_wx, dg_Wh = _split_gcn_w(np.asarray(inputs['dec_Wg'], np.float32), 2 * H)
    du_wx, du_Wh = _split_gcn_w(np.asarray(inputs['dec_Wu'], np.float32), H)
    bg_e = np.asarray(inputs['enc_bg'], np.float32)
    bu_e = np.asarray(inputs['enc_bu'], np.float32)
    bg_d = np.asarray(inputs['dec_bg'], np.float32)
    bu_d = np.asarray(inputs['dec_bu'], np.float32)
    wp = np.asarray(inputs['proj_W'], np.float32)[:, 0]  # [H]
    bp = float(np.asarray(inputs['proj_b'], np.float32)[0])

    c = {}
    # G_k^T tiles for the big matmuls: gt[p, k, jc, i] = G_{k+1}[i, jc*128+p]
    gt = np.empty((128, 2, IC, N), np.float32)
    for k in range(2):
        Gk = G[k + 1]                      # [i, j]
        # gt[p, k, jc, i] = Gk[i, jc*128+p]
        gt[:, k] = Gk.T.reshape(IC, 128, N).transpose(1, 0, 2)
    c['gt'] = gt.astype(bf)

    # g1 rows (G_k @ ones), broadcast over batch: [2, IC*BB*128]
    g1 = G[1:].sum(axis=2)                 # [2, N]
    g1st = np.broadcast_to(
        g1.reshape(2, IC, 1, 128), (2, IC, BB, 128)).reshape(2, IC * BB * 128)
    c['g1st'] = np.ascontiguousarray(g1st).astype(bf)

    def vs(*rows):
        return np.ascontiguousarray(np.vstack(rows)).astype(bf)

    z64 = np.zeros((64, 2 * H), np.float32)
    # ---- encoder gates: lhsT rows [hT(64); S1hT(64)] and [S2hT(64); ones;
    #      xT; Xs1T; Xs2T]
    c['enc_g_a'] = vs(eg_Wh[0], eg_Wh[1])                      # [128, 128]
    c['enc_g_b'] = vs(eg_Wh[2], bg_e[None],
                      eg_wx[0][None], eg_wx[1][None], eg_wx[2][None])  # [68,128]
    # ---- encoder update: S2a rows [rhT; U1hT], S2b rows [U2hT; ones],
    #      STb rows [S2hT(zero-weight); ones(zero); xT; Xs1T; Xs2T]
    c['enc_u_a'] = vs(eu_Wh[0], eu_Wh[1])                      # [128, 64]
    c['enc_u_b'] = vs(eu_Wh[2], bu_e[None])                    # [65, 64]
    c['enc_u_x'] = vs(np.zeros((65, H), np.float32),
                      eu_wx[0][None], eu_wx[1][None], eu_wx[2][None])  # [68, 64]
    # ---- decoder step 0 (y = 0)
    c['dec0_g_a'] = vs(dg_Wh[0], dg_Wh[1])
    c['dec0_g_b'] = vs(dg_Wh[2], bg_d[None], np.zeros((2, 2 * H), np.float32))
    c['dec0_u_a'] = vs(du_Wh[0], du_Wh[1])
    c['dec0_u_b'] = vs(du_Wh[2], bu_d[None])
    # ---- decoder steps >= 1: y = h @ Wp + bp folded in; extra y output col
    c['dec_g_a'] = vs(dg_Wh[0] + np.outer(wp, dg_wx[0]),
                      dg_Wh[1] + np.outer(wp, dg_wx[1]))
    c['dec_g_b'] = vs(dg_Wh[2] + np.outer(wp, dg_wx[2]),
                      (bg_d + bp * dg_wx[0])[None],
                      (bp * dg_wx[1])[None], (bp * dg_wx[2])[None])  # [67,128]

    def ycol(mat, col):
        return np.concatenate([mat, col[:, None]], axis=1)

    c['dec_u_a'] = vs(ycol(np.vstack([du_Wh[0], du_Wh[1]]),
                           np.zeros(128, np.float32)))          # [128, 65]
    ub = np.vstack([du_Wh[2], (bu_d + bp * du_wx[0])[None]])    # [65, 64]
    ubc = np.zeros(65, np.float32); ubc[64] = bp
    c['dec_u_b'] = vs(ycol(ub, ubc))                            # [65, 65]
    uga = np.vstack([np.outer(wp, du_wx[0]), np.outer(wp, du_wx[1])])
    ugac = np.concatenate([wp, np.zeros(64, np.float32)])
    c['dec_u_ga'] = vs(ycol(uga, ugac))                         # [128, 65]
    ugb = np.vstack([np.outer(wp, du_wx[2]), np.zeros((1, H), np.float32),
                     (bp * du_wx[1])[None], (bp * du_wx[2])[None]])
    c['dec_u_gb'] = vs(ycol(ugb, np.zeros(67, np.float32)))     # [67, 65]
    # phantom y emission
    c['wp_col'] = np.ascontiguousarray(wp[:, None]).astype(bf)  # [64, 1]
    c['bp'] = bp
    return c


def _prep_xst(inputs):
    """Per-core runtime inputs (bf16):
    xrow [T, IC*BB*128]: x^T rows, xrow[t, (ic b n)] = x[b, t, ic*128+n]
    xq   [128, IC, T*BB]: node-major x, xq[p, jc, (t b)] = x[b, t, jc*128+p]
    (G1 x)^T / (G2 x)^T rows are computed on device from xq."""
    import ml_dtypes
    bf = ml_dtypes.bfloat16
    x = np.asarray(inputs['x'], np.float32)[..., 0]   # [B, T, N]
    out = []
    for cix in range(M):
        xc = x[cix * BB:(cix + 1) * BB]               # [BB, T, N]
        xrow = xc.reshape(BB, T, IC, 128).transpose(1, 2, 0, 3)
        xrow = np.ascontiguousarray(xrow.reshape(T, IC * BB * 128)).astype(bf)
        xq = xc.reshape(BB, T, IC, 128).transpose(3, 2, 1, 0)  # [128,IC,T,BB]
        xq = np.ascontiguousarray(xq.reshape(128, IC, T * BB)).astype(bf)
        out.append({"xrow": xrow, "xq": xq})
    return out


# ------------------------------------------------------------ program build

def _build_program(consts):
    import concourse.bass as bass
    import concourse.tile as tile
    from concourse import bacc, mybir
    from concourse.masks import make_identity
    from contextlib import ExitStack

    F32 = mybir.dt.float32
    BF16 = mybir.dt.bfloat16
    AF = mybir.ActivationFunctionType

    nc = bacc.Bacc("TRN2", target_bir_lowering=False, debug=False)
    xrow_d = nc.dram_tensor("xrow", [T, IC * BB * 128], BF16,
                            kind="ExternalInput")
    xq_d = nc.dram_tensor("xq", [128, IC, T * BB], BF16, kind="ExternalInput")
    y_d = nc.dram_tensor("y", [HOR * BB, N], BF16, kind="ExternalOutput")
    cd = {k: nc.inline_tensor(v, f"c_{k}") for k, v in consts.items()
          if isinstance(v, np.ndarray)}
    bp = consts['bp']

    with tile.TileContext(nc) as tc, ExitStack() as ctx:
        cpool = ctx.enter_context(tc.tile_pool(name="const", bufs=1))
        spool = ctx.enter_context(tc.tile_pool(name="state", bufs=1))
        wpool = ctx.enter_context(tc.tile_pool(name="work", bufs=3))
        psA = ctx.enter_context(tc.tile_pool(name="psA", bufs=4, space="PSUM"))
        psB = ctx.enter_context(tc.tile_pool(name="psB", bufs=2, space="PSUM"))
        psT = ctx.enter_context(tc.tile_pool(name="psT", bufs=2, space="PSUM"))
        dpool = ctx.enter_context(tc.tile_pool(name="dram", bufs=1,
                                               space="DRAM"))

        # ---- persistent tiles
        gt = cpool.tile([128, 2, IC, N], BF16)
        g1s = cpool.tile([2, IC * BB * 128], BF16)
        ident = cpool.tile([128, 128], BF16)
        identf = cpool.tile([128, 128], F32)
        W = {}
        for k, v in consts.items():
            if isinstance(v, np.ndarray) and k not in ('gt', 'g1st'):
                W[k] = cpool.tile(list(v.shape), BF16, name=f"w_{k}")
                nc.sync.dma_start(W[k][:], cd[k].ap())
        nc.sync.dma_start(gt[:], cd['gt'].ap())
        nc.sync.dma_start(g1s[:], cd['g1st'].ap())
        make_identity(nc, ident[:])
        make_identity(nc, identf[:])

        ha = spool.tile([128, IC, BB, H], F32)     # h master
        hb = spool.tile([128, IC, BB, H], BF16)    # h bf16 (matmul stationary)
        rhb = spool.tile([128, IC, BB, H], BF16)   # r*h bf16
        STa = spool.tile([128, IC, BB, 128], BF16)  # rows 0:64 hT, 64:128 S1hT
        STb = spool.tile([68, IC, BB, 128], BF16)   # 0:64 S2hT, 64 ones, 65:68 x
        S2a = spool.tile([128, IC, BB, 128], BF16)  # 0:64 rhT, 64:128 U1hT
        S2b = spool.tile([65, IC, BB, 128], BF16)   # 0:64 U2hT, 64 ones
        zr = spool.tile([128, IC, BB, 2 * H], F32)
        yall = spool.tile([128, IC, HOR, BB], F32)
        yT = spool.tile([HOR * BB, N], BF16)
        xq_sb = spool.tile([128, IC, T * BB], BF16)
        xs_sb = spool.tile([T * BB, 2, 2, 512], BF16)
        xs_d = dpool.tile([T, 2, IC * BB * 128], BF16)

        for t_ in (ha, hb, STa, S2a):
            nc.gpsimd.memset(t_[:], 0.0)
        nc.gpsimd.memset(STb[0:64, :, :, :], 0.0)
        nc.gpsimd.memset(S2b[0:64, :, :, :], 0.0)
        nc.gpsimd.memset(STb[64:68, :, :, :], 0.0)
        nc.gpsimd.memset(STb[64:65, :, :, :], 1.0)
        nc.gpsimd.memset(S2b[64:65, :, :, :], 1.0)

        def bigmm(stat, dst_hi, dst_lo):
            """S_k^T = (G_k @ v)^T for k=1,2 into dst rows (bf16)."""
            for ih in range(2):
                for bp_ in range(4):
                    for k in range(2):
                        ps = psA.tile([128, 512], F32, tag="big")
                        for jc in range(IC):
                            nc.tensor.matmul(
                                ps[:],
                                stat[:, jc, 2 * bp_:2 * bp_ + 2, :],
                                gt[:, k, jc, ih * 512:(ih + 1) * 512],
                                start=(jc == 0), stop=(jc == IC - 1))
                        dst = dst_hi if k == 0 else dst_lo
                        rb = 64 if k == 0 else 0
                        for par in range(2):
                            b = 2 * bp_ + par
                            src = ps[par * 64:(par + 1) * 64, :].rearrange(
                                "p (a n) -> p a n", n=128)
                            # nc.any: let the scheduler route each drain to
                            # whichever of ACT/DVE is idle (measured better
                            # than any static split)
                            nc.any.tensor_copy(
                                dst[rb:rb + 64, ih * 4:(ih + 1) * 4, b, :], src)

        def stage2(mms, cols, out_cb):
            """mms: list of (lhsT_fn(ic,b) -> AP, W tile). out [128, 4*cols]
            per (ic, bgroup); out_cb(ic, bg, ps)."""
            for bg in range(2):
                for ic in range(IC):
                    ps = psB.tile([128, 4 * cols], F32, tag="s2")
                    for bi in range(4):
                        b = bg * 4 + bi
                        o = ps[:, bi * cols:(bi + 1) * cols]
                        for i, (lf, Wt) in enumerate(mms):
                            nc.tensor.matmul(o, lf(ic, b), Wt[:],
                                             start=(i == 0),
                                             stop=(i == len(mms) - 1))
                    out_cb(ic, bg, ps)

        def transpose_to(src, dst, dst_rb):
            """src [128, IC, BB, 64] bf16 -> dst[dst_rb:dst_rb+64] rows (f on
            partitions), per (ic, bgroup)."""
            for bg in range(2):
                for ic in range(IC):
                    pst = psT.tile([128, 256], BF16, tag="s2")
                    for bl in range(2):
                        b0 = bg * 4 + 2 * bl
                        nc.tensor.transpose(
                            pst[:, bl * 128:(bl + 1) * 128],
                            src[:, ic, b0:b0 + 2, :], ident[:])
                    for par in range(2):
                        b0 = bg * 4 + par
                        s = pst[par * 64:(par + 1) * 64, :].rearrange(
                            "p (a n) -> p a n", n=128)
                        nc.any.tensor_copy(
                            dst[dst_rb:dst_rb + 64, ic, b0:b0 + 3:2, :], s)

        def gates_cb(ic, bg, ps):
            nc.scalar.activation(zr[:, ic, bg * 4:bg * 4 + 4, :], ps[:],
                                 AF.Sigmoid)
            nc.vector.tensor_tensor(
                rhb[:, ic, bg * 4:bg * 4 + 4, :],
                zr[:, ic, bg * 4:bg * 4 + 4, H:2 * H],
                ha[:, ic, bg * 4:bg * 4 + 4, :],
                mybir.AluOpType.mult)

        def update_cb_factory(cols, d):
            def update_cb(ic, bg, ps):
                bs = slice(bg * 4, bg * 4 + 4)
                pv = ps.rearrange("p (a c) -> p a c", c=cols)
                nt = wpool.tile([128, 4, H], F32, tag="nt")
                nc.scalar.activation(nt[:], pv[:, :, 0:H], AF.Tanh)
                if cols == 65:
                    nc.any.tensor_copy(yall[:, ic, d - 1, bs], pv[:, :, 64])
                dt_ = wpool.tile([128, 4, H], F32, tag="dt")
                nc.vector.tensor_tensor(dt_[:], nt[:], ha[:, ic, bs, :],
                                        mybir.AluOpType.subtract)
                mt = wpool.tile([128, 4, H], F32, tag="mt")
                nc.vector.tensor_tensor(mt[:], zr[:, ic, bs, 0:H], dt_[:],
                                        mybir.AluOpType.mult)
                nc.vector.tensor_tensor(ha[:, ic, bs, :], ha[:, ic, bs, :],
                                        mt[:], mybir.AluOpType.add)
                nc.any.tensor_copy(hb[:, ic, bs, :], ha[:, ic, bs, :])
            return update_cb

        def step(g_mms, u_mms, ucols, d):
            # h' of the previous step is transposed AFTER this step's first
            # big-matmul phase is emitted: the PE chews on bigmm (27us) while
            # ACT/DVE finish the previous step's h' elementwise chain, instead
            # of stalling on it at the step boundary.
            bigmm(hb, STa, STb)
            transpose_to(hb, STa, 0)
            stage2(g_mms, 2 * H, gates_cb)
            transpose_to(rhb, S2a, 0)
            bigmm(rhb, S2a, S2b)
            stage2(u_mms, ucols, update_cb_factory(ucols, d))

        # ---------------- on-device (G_k @ x)^T rows for all encoder steps
        nc.sync.dma_start(xq_sb[:], xq_d.ap())
        for k in range(2):
            for ih in range(2):
                ps = psA.tile([T * BB, 512], F32, tag="big", name="xps")
                for jc in range(IC):
                    nc.tensor.matmul(ps[:], xq_sb[:, jc, :],
                                     gt[:, k, jc, ih * 512:(ih + 1) * 512],
                                     start=(jc == 0), stop=(jc == IC - 1))
                nc.any.tensor_copy(xs_sb[:, k, ih, :], ps[:])
        for k in range(2):
            for ih in range(2):
                # xs_d[t, k, (ic b n)] with ic = ih*4+icl; DMA APs balance
                # only up to 3 dims, so split over t
                dst4 = xs_d[:, k, :].rearrange(
                    "t (i b n) -> t i b n", b=BB, n=128)[
                        :, ih * 4:(ih + 1) * 4, :, :].rearrange(
                        "t i b n -> t b i n")
                src4 = xs_sb[:, k, ih, :].rearrange(
                    "(t b) (i n) -> t b i n", b=BB, n=128)
                for t in range(T):
                    nc.sync.dma_start(dst4[t], src4[t])

        # ---------------- encoder
        for t in range(T):
            nc.sync.dma_start(STb[65:66, :, :, :].rearrange(
                "p a b n -> p (a b n)"), xrow_d.ap()[t:t + 1])
            nc.sync.dma_start(STb[66:68, :, :, :].rearrange(
                "p a b n -> p (a b n)"), xs_d[t])
            g_mms = [(lambda ic, b: STa[:, ic, b, :], W['enc_g_a']),
                     (lambda ic, b: STb[0:68, ic, b, :], W['enc_g_b'])]
            u_mms = [(lambda ic, b: S2a[:, ic, b, :], W['enc_u_a']),
                     (lambda ic, b: S2b[0:65, ic, b, :], W['enc_u_b']),
                     (lambda ic, b: STb[0:68, ic, b, :], W['enc_u_x'])]
            step(g_mms, u_mms, H, 0)

        # ---------------- decoder
        nc.sync.dma_start(STb[65:67, :, :, :].rearrange(
            "p a b n -> p (a b n)"), g1s[:])
        for d in range(HOR):
            if d == 0:
                g_mms = [(lambda ic, b: STa[:, ic, b, :], W['dec0_g_a']),
                         (lambda ic, b: STb[0:67, ic, b, :], W['dec0_g_b'])]
                u_mms = [(lambda ic, b: S2a[:, ic, b, :], W['dec0_u_a']),
                         (lambda ic, b: S2b[0:65, ic, b, :], W['dec0_u_b'])]
                step(g_mms, u_mms, H, d)
            else:
                g_mms = [(lambda ic, b: STa[:, ic, b, :], W['dec_g_a']),
                         (lambda ic, b: STb[0:67, ic, b, :], W['dec_g_b'])]
                u_mms = [(lambda ic, b: S2a[:, ic, b, :], W['dec_u_a']),
                         (lambda ic, b: S2b[0:65, ic, b, :], W['dec_u_b']),
                         (lambda ic, b: STa[:, ic, b, :], W['dec_u_ga']),
                         (lambda ic, b: STb[0:67, ic, b, :], W['dec_u_gb'])]
                step(g_mms, u_mms, H + 1, d)

        # phantom step: emit y_12 = h_12 @ Wp + bp
        transpose_to(hb, STa, 0)
        for ic in range(IC):
            ps = psB.tile([128, BB], F32, tag="s2")
            for b in range(BB):
                nc.tensor.matmul(ps[:, b:b + 1], STa[0:64, ic, b, :],
                                 W['wp_col'][:], start=True, stop=True)
            nc.scalar.activation(yall[:, ic, HOR - 1, :], ps[:],
                                 AF.Identity, bias=bp)

        # transpose y to [(t,b), n] and store
        for ic in range(IC):
            pst = psB.tile([HOR * BB, 128], F32, tag="s2")
            nc.tensor.transpose(pst[:], yall[:, ic, :, :].rearrange(
                "p a b -> p (a b)"), identf[:])
            nc.any.tensor_copy(yT[:, ic * 128:(ic + 1) * 128], pst[:])
        nc.sync.dma_start(y_d.ap(), yT[:])

    nc.compile()
    return nc


def _make_runner(nc):
    """Build a persistently-jitted executor for the program (the stock
    run_bass_kernel_spmd path re-jits a fresh closure every call)."""
    import jax
    import jax.numpy as jnp
    from jax.experimental.shard_map import shard_map
    from jax.sharding import Mesh, PartitionSpec, NamedSharding
    from concourse import bass2jax, mybir

    bass2jax.install_neuronx_cc_hook()
    assert nc.dbg_addr is None
    pid_name = (nc.partition_id_tensor.name
                if nc.partition_id_tensor is not None else None)

    in_names, out_names, out_avals, zero_shapes = [], [], [], []
    for alloc in nc.m.functions[0].allocations:
        if not isinstance(alloc, mybir.MemoryLocationSet):
            continue
        name = alloc.memorylocations[0].name
        if alloc.kind == "ExternalInput":
            if name != pid_name:
                in_names.append(name)
        elif alloc.kind == "ExternalOutput":
            out_names.append(name)
            shape = tuple(alloc.tensor_shape)
            dtype = mybir.dt.np(alloc.dtype)
            out_avals.append(jax.core.ShapedArray(shape, dtype))
            zero_shapes.append((shape, dtype))
    n_params = len(in_names)
    all_names = list(in_names + out_names)
    if pid_name is not None:
        all_names.append(pid_name)
    all_names = tuple(all_names)

    def _body(*args):
        operands = list(args)
        if pid_name is not None:
            operands.append(bass2jax.partition_id_tensor())
        outs = bass2jax._bass_exec_p.bind(
            *operands,
            out_avals=tuple(out_avals),
            in_names=all_names,
            out_names=tuple(out_names),
            lowering_input_output_aliases=(),
            sim_require_finite=True,
            sim_require_nnan=True,
            nc=nc,
        )
        return tuple(outs)

    devices = jax.devices()[:M]
    mesh = Mesh(np.asarray(devices), ("core",))
    spec = PartitionSpec("core")
    n_outs = len(out_names)
    fn = jax.jit(
        shard_map(_body, mesh=mesh,
                  in_specs=(spec,) * (n_params + n_outs),
                  out_specs=(spec,) * n_outs, check_rep=False),
        donate_argnums=tuple(range(n_params, n_params + n_outs)),
        keep_unused=True)
    sh = NamedSharding(mesh, spec)
    zfns = [jax.jit(lambda s=s, d=d: jnp.zeros((M * s[0],) + s[1:], d),
                    out_shardings=sh) for (s, d) in zero_shapes]

    def run(per_core_inputs):
        """per_core_inputs: list (per core) of dict name->np.ndarray.
        Returns list of dict name->np.ndarray."""
        cat = [np.concatenate([np.asarray(per_core_inputs[c][nm])
                               for c in range(M)], axis=0)
               for nm in in_names]
        outs = fn(*cat, *[z() for z in zfns])
        res = []
        for c in range(M):
            d = {}
            for i, nm in enumerate(out_names):
                a = np.asarray(outs[i])
                d[nm] = a.reshape(M, a.shape[0] // M, *a.shape[1:])[c]
            res.append(d)
        return res

    return run


# ------------------------------------------------------------------ fallback

def _kernel_jax(inputs):
    import jax
    import jax.numpy as jnp
    from functools import partial

    def _gcn(G, x, Wt, b):
        bb, nn, cc = x.shape
        sup = jnp.einsum('kij,bjc->bikc', G, x)
        return sup.reshape(bb, nn, -1) @ Wt + b

    def _cell(G, x_t, h, Wg, bg, Wu, bu):
        comb = jnp.concatenate([x_t, h], axis=-1)
        z, r = jnp.split(jax.nn.sigmoid(_gcn(G, comb, Wg, bg)), 2, axis=-1)
        n = jnp.tanh(_gcn(G, jnp.concatenate([x_t, r * h], axis=-1), Wu, bu))
        return z * n + (1.0 - z) * h

    @partial(jax.pmap, axis_name='i',
             in_axes=(0,) + (None,) * 11)
    def _run(x, G, enc_Wg, enc_bg, enc_Wu, enc_bu,
             dec_Wg, dec_bg, dec_Wu, dec_bu, proj_W, proj_b):
        bb = x.shape[0]
        h0 = jnp.zeros((bb, N, H), x.dtype)

        def enc_step(h, x_t):
            return _cell(G, x_t, h, enc_Wg, enc_bg, enc_Wu, enc_bu), None

        h, _ = jax.lax.scan(enc_step, h0, x.transpose(1, 0, 2, 3))
        y0 = jnp.zeros((bb, N, C), x.dtype)

        def dec_step(carry, _):
            h, y = carry
            h = _cell(G, y, h, dec_Wg, dec_bg, dec_Wu, dec_bu)
            out = h @ proj_W + proj_b
            return (h, out), out

        _, outs = jax.lax.scan(dec_step, (h, y0), None, length=HOR)
        return outs.transpose(1, 0, 2, 3)

    import jax.numpy as jnp
    x = np.asarray(inputs['x'], dtype=np.float32)
    xs = jnp.asarray(x.reshape(M, B // M, T, N, C))
    args = tuple(jnp.asarray(np.asarray(inputs[k], dtype=np.float32)) for k in
                 ('G', 'enc_Wg', 'enc_bg', 'enc_Wu', 'enc_bu',
                  'dec_Wg', 'dec_bg', 'dec_Wu', 'dec_bu', 'proj_W', 'proj_b'))
    out = _run(xs, *args)
    return np.asarray(out).reshape(B, HOR, N, C).astype(np.float32)


# ------------------------------------------------------------------- kernel

def _structure_ok(inputs):
    try:
        x = np.asarray(inputs['x'])
        G = np.asarray(inputs['G'])
        if x.shape != (B, T, N, C) or G.shape != (K, N, N):
            return False
        return bool(np.abs(G[0] - np.eye(N, dtype=np.float32)).max() < 1e-6)
    except Exception:
        return False


def kernel(**inputs):
    global _BUILT
    if not _structure_ok(inputs):
        return _kernel_jax(inputs)
    try:
        import hashlib
        hsh = hashlib.sha256()
        for k in ('G', 'enc_Wg', 'enc_bg', 'enc_Wu', 'enc_bu', 'dec_Wg',
                  'dec_bg', 'dec_Wu', 'dec_bu', 'proj_W', 'proj_b'):
            hsh.update(np.ascontiguousarray(
                np.asarray(inputs[k], np.float32)).tobytes())
        key = hsh.hexdigest()
        if _BUILT is None or _BUILT[0] != key:
            consts = _prep_consts(inputs)
            nc = _build_program(consts)
            _BUILT = (key, _make_runner(nc))
        run = _BUILT[1]
        xsts = _prep_xst(inputs)
        res = run(xsts)
        out = np.empty((B, HOR, N, C), np.float32)
        for cix in range(M):
            yc = np.asarray(res[cix]["y"], np.float32).reshape(HOR, BB, N)
            out[cix * BB:(cix + 1) * BB] = \
                yc.transpose(1, 0, 2).reshape(BB, HOR, N, C)
        return out
    except Exception:
        _BUILT = None
        return _kernel_jax(inputs)
